# revision 6
# baseline (speedup 1.0000x reference)
"""CRF Viterbi decode kernel for Trainium2 (8 NeuronCores, data-parallel over batch).

Problem: emissions [1024, 1024, 20] f32, transitions [20, 20], start/end [20]
-> best tag sequence [1024, 1024] int32 (torchcrf.CRF.decode semantics,
exact f32 arithmetic matching the jax reference, first-index argmax ties).

Sharding: batch 1024 / 8 cores = 128 sequences per core = 128 SBUF partitions.
Everything on the Vector engine (DVE), raw bass (no TileContext): a single
engine executes in order, so no semaphores are needed between compute ops.

Forward (per step s, all [128, x] DVE ops):
  cand[b,(j,i)] = score[b,i] + transT[(j,i)]         TT add (broadcast AP)
  best[b,j]     = max_i cand                          TensorReduce max axis=X
  score'[b,j]   = best + emissions[b,s,j]             TT add [128,20]
  mask          = (cand == best_bcast)                TT is_equal
  mv            = mask * revIota  (rev = 19-i)        TT mult
  bp[b,s,j]     = max_i mv   (= 19 - argmax, first-index ties)  TR max
Backward (per step): one scalar_tensor_tensor:
  revtag[s-1] = sum_j (revJ == revtag[s]) * bp_s[j]   (accum_out)
Decode: tags = 19 - revtag, cast int32, DMA out.
"""

import sys

for _p in ("/opt/trn_rl_repo",):
    if _p not in sys.path:
        sys.path.insert(0, _p)

import numpy as np

B, S, T = 1024, 1024, 20
NCORES = 8
PB = B // NCORES  # 128 sequences per core
F = T * T  # 400
REV_MAX = T - 1  # 19

_CACHE = {}


def _build_nc():
    import concourse.bass as bass
    import concourse.mybir as mybir

    nc = bass.Bass("TRN2", debug=False, num_devices=NCORES)
    f32 = mybir.dt.float32
    i32 = mybir.dt.int32
    add = mybir.AluOpType.add
    amax = mybir.AluOpType.max
    aeq = mybir.AluOpType.is_equal
    amult = mybir.AluOpType.mult
    X = mybir.AxisListType.X

    # ---- DRAM I/O ----
    # consts layout (f32 cols): transT[0:400] revIotaF[400:800] revJ20[800:820]
    #                           start[820:840] end[840:860]
    NC_CONST = F + F + T + T + T
    em_d = nc.dram_tensor("em", [PB, S, T], f32, kind="ExternalInput").ap()
    cst_d = nc.dram_tensor("cst", [PB, NC_CONST], f32, kind="ExternalInput").ap()
    out_d = nc.dram_tensor("out", [PB, S], i32, kind="ExternalOutput").ap()

    # ---- SBUF ----
    def sb(name, shape, dt):
        return nc.alloc_sbuf_tensor(name, shape, dt).ap()

    em_t = sb("em_sb", [PB, S * T], f32)          # 80 KB/partition
    cst_t = sb("cst_sb", [PB, NC_CONST], f32)
    bp_t = sb("bp_sb", [PB, (S - 1) * T], f32)    # 80 KB/partition (bp for s=1..S-1)
    cand_t = sb("cand_sb", [PB, F], f32)
    mask_t = sb("mask_sb", [PB, F], f32)
    mv_t = sb("mv_sb", [PB, F], f32)
    score_a = sb("score_a_sb", [PB, T], f32)
    score_b = sb("score_b_sb", [PB, T], f32)
    best_t = sb("best_sb", [PB, T], f32)
    fs_t = sb("fs_sb", [PB, T], f32)
    fbest_t = sb("fbest_sb", [PB, 1], f32)
    revtag_t = sb("revtag_sb", [PB, S], f32)
    seltrash_t = sb("seltrash_sb", [PB, T], f32)
    tags_t = sb("tags_sb", [PB, S], i32)

    transT_v = cst_t[:, 0:F]
    revIotaF_v = cst_t[:, F : 2 * F]
    revJ_v = cst_t[:, 2 * F : 2 * F + T]
    start_v = cst_t[:, 2 * F + T : 2 * F + 2 * T]
    end_v = cst_t[:, 2 * F + 2 * T : 2 * F + 3 * T]

    V = nc.vector

    # ---- load inputs ----
    dma_sem = nc.alloc_semaphore()
    nc.sync.dma_start(em_t[:], em_d.rearrange("b s t -> b (s t)")).then_inc(dma_sem, 16)
    nc.sync.dma_start(cst_t[:], cst_d[:]).then_inc(dma_sem, 16)
    V.wait_ge(dma_sem, 32)

    cand3 = cand_t[:].rearrange("p (j i) -> p j i", j=T)
    mask3 = mask_t[:].rearrange("p (j i) -> p j i", j=T)
    mv3 = mv_t[:].rearrange("p (j i) -> p j i", j=T)
    transT3 = transT_v.rearrange("p (j i) -> p j i", j=T)
    revIotaF3 = revIotaF_v.rearrange("p (j i) -> p j i", j=T)

    # ---- forward ----
    # score_0 = start + em[:, 0, :]
    V.tensor_tensor(score_a[:], em_t[:, 0:T], start_v, op=add)
    V.drain()  # first loop iteration broadcasts score_a immediately

    cur, nxt = score_a, score_b
    for s in range(1, S):
        score_bc = cur[:].unsqueeze(1).broadcast_to([PB, T, T])
        V.tensor_tensor(cand3, score_bc, transT3, op=add)
        V.drain()
        V.tensor_reduce(best_t[:], cand3, axis=X, op=amax)
        V.drain()
        V.tensor_tensor(nxt[:], best_t[:], em_t[:, s * T : (s + 1) * T], op=add)
        V.drain()
        best_bc = best_t[:].unsqueeze(2).broadcast_to([PB, T, T])
        V.tensor_tensor(mask3, cand3, best_bc, op=aeq)
        V.drain()
        V.tensor_tensor(mv3, mask3, revIotaF3, op=amult)
        V.drain()
        V.tensor_reduce(bp_t[:, (s - 1) * T : s * T], mv3, axis=X, op=amax)
        V.drain()
        cur, nxt = nxt, cur

    # ---- final argmax: fs = score + end ----
    V.tensor_tensor(fs_t[:], cur[:], end_v, op=add)
    V.drain()
    V.tensor_reduce(fbest_t[:], fs_t[:], axis=X, op=amax)
    V.drain()
    fbest_bc = fbest_t[:].broadcast_to([PB, T])
    V.tensor_tensor(mask_t[:, 0:T], fs_t[:], fbest_bc, op=aeq)
    V.drain()
    V.tensor_tensor(mv_t[:, 0:T], mask_t[:, 0:T], revJ_v, op=amult)
    V.drain()
    V.tensor_reduce(revtag_t[:, S - 1 : S], mv_t[:, 0:T], axis=X, op=amax)

    # ---- backward: revtag[s-1] = sum_j (revJ == revtag[s]) * bp_s[j] ----
    for s in range(S - 1, 0, -1):
        V.drain()  # scalar operand revtag[:, s] written by the previous step
        V.scalar_tensor_tensor(
            seltrash_t[:],
            revJ_v,
            revtag_t[:, s : s + 1],
            bp_t[:, (s - 1) * T : s * T],
            op0=aeq,
            op1=amult,
            accum_out=revtag_t[:, s - 1 : s],
        )

    # ---- decode: tags = 19 - revtag  (= revtag * -1 + 19), cast to int32 ----
    V.drain()
    V.tensor_scalar(
        tags_t[:], revtag_t[:], -1.0, float(REV_MAX), op0=amult, op1=add
    )

    # ---- store ----
    nc.all_engine_barrier()
    nc.sync.dma_start(out_d[:], tags_t[:]).then_inc(dma_sem, 16)
    for eng in nc.engines.values():
        eng.wait_ge(dma_sem, 48)

    return nc


def _get_compiled():
    if "nc" not in _CACHE:
        _CACHE["nc"] = _build_nc()
    return _CACHE["nc"]


def _make_consts(start_transitions, end_transitions, transitions):
    transT = np.ascontiguousarray(transitions.astype(np.float32).T).reshape(1, F)
    revIotaF = np.tile(
        (REV_MAX - np.arange(T, dtype=np.float32)), T
    ).reshape(1, F)
    revJ = (REV_MAX - np.arange(T, dtype=np.float32)).reshape(1, T)
    cst = np.concatenate(
        [
            transT,
            revIotaF,
            revJ,
            start_transitions.astype(np.float32).reshape(1, T),
            end_transitions.astype(np.float32).reshape(1, T),
        ],
        axis=1,
    )
    return np.ascontiguousarray(np.broadcast_to(cst, (PB, cst.shape[1])))


def kernel(emissions, start_transitions, end_transitions, transitions):
    from concourse.bass_utils import run_bass_kernel_spmd

    emissions = np.asarray(emissions, dtype=np.float32)
    cst = _make_consts(
        np.asarray(start_transitions),
        np.asarray(end_transitions),
        np.asarray(transitions),
    )

    nc = _get_compiled()
    in_maps = []
    for c in range(NCORES):
        in_maps.append(
            {
                "em": np.ascontiguousarray(emissions[c * PB : (c + 1) * PB]),
                "cst": cst,
            }
        )
    res = run_bass_kernel_spmd(nc, in_maps, core_ids=list(range(NCORES)))
    out = np.concatenate([r["out"] for r in res.results], axis=0)
    return out.astype(np.int32)


# revision 11
# speedup vs baseline: 7.4691x; 7.4691x over previous
"""CRF Viterbi decode kernel for Trainium2 (8 NeuronCores, data-parallel over batch).

emissions [1024,1024,20] f32 + transitions -> best tag path [1024,1024] int32,
bit-exact with the jax reference (same f32 op order, first-index argmax ties).

Environment characteristics (measured): every engine instruction costs ~29us
regardless of size; engines do not overlap; an explicit DVE drain (~10us) is
required between a producer and a dependent consumer except for the TT->TR
pair. The design therefore minimizes instruction count:

  forward chain (per step, sequential):  TT cand / TR best / drain /
                                         TT score' / drain            (5 instr)
  backpointer extraction: recomputed in batches of K steps from the stored
    score history (5 ops + 3 drains per K steps) - bit-identical recompute.
  backward: one fused select-accumulate STT + drain per step.
"""

import sys

for _p in ("/opt/trn_rl_repo", "/root/.axon_site/_ro/trn_rl_repo"):
    import os as _os

    if _os.path.isdir(_p) and _p not in sys.path:
        sys.path.insert(0, _p)

import numpy as np

B, S, T = 1024, 1024, 20
NCORES = 8
PB = B // NCORES  # 128
F = T * T  # 400
REV_MAX = T - 1  # 19
KEXT = 16  # extraction batch size (steps)

_CACHE = {}


def _build_nc(n_steps=None, phases=7):
    import concourse.bass as bass
    import concourse.mybir as mybir

    if n_steps is None:
        n_steps = S
    nc = bass.Bass("TRN2", debug=False, num_devices=NCORES)
    f32 = mybir.dt.float32
    i32 = mybir.dt.int32
    add = mybir.AluOpType.add
    amax = mybir.AluOpType.max
    aeq = mybir.AluOpType.is_equal
    amult = mybir.AluOpType.mult
    X = mybir.AxisListType.X

    NC_CONST = F + F + T + T + T
    em_d = nc.dram_tensor("em", [PB, S, T], f32, kind="ExternalInput").ap()
    cst_d = nc.dram_tensor("cst", [PB, NC_CONST], f32, kind="ExternalInput").ap()
    out_d = nc.dram_tensor("out", [PB, S], i32, kind="ExternalOutput").ap()

    def sb(name, shape, dt):
        return nc.alloc_sbuf_tensor(name, shape, dt).ap()

    em_t = sb("em_sb", [PB, S * T], f32)            # 80 KB/partition
    cst_t = sb("cst_sb", [PB, NC_CONST], f32)
    scores_t = sb("scores_sb", [PB, S * T], f32)    # 80 KB/partition: score_s at col s*T
    cand_t = sb("cand_sb", [PB, F], f32)
    best_t = sb("best_sb", [PB, T], f32)
    candB_t = sb("candB_sb", [PB, KEXT * F], f32)   # extraction batch (in-place reuse)
    bestB_t = sb("bestB_sb", [PB, KEXT * T], f32)
    bp_t = scores_t  # bp for step s overwrites score col (s-1) after extraction
    fs_t = sb("fs_sb", [PB, T], f32)
    fbest_t = sb("fbest_sb", [PB, 1], f32)
    revtag_t = sb("revtag_sb", [PB, S], f32)
    seltrash_t = sb("seltrash_sb", [PB, T], f32)
    mv20_t = sb("mv20_sb", [PB, T], f32)
    tags_t = sb("tags_sb", [PB, S], i32)

    transT_v = cst_t[:, 0:F]
    revIotaF_v = cst_t[:, F : 2 * F]
    revJ_v = cst_t[:, 2 * F : 2 * F + T]
    start_v = cst_t[:, 2 * F + T : 2 * F + 2 * T]
    end_v = cst_t[:, 2 * F + 2 * T : 2 * F + 3 * T]

    V = nc.vector

    dma_sem = nc.alloc_semaphore()
    nc.sync.dma_start(em_t[:], em_d.rearrange("b s t -> b (s t)")).then_inc(dma_sem, 16)
    nc.sync.dma_start(cst_t[:], cst_d[:]).then_inc(dma_sem, 16)
    V.wait_ge(dma_sem, 32)

    cand3 = cand_t[:].rearrange("p (j i) -> p j i", j=T)
    transT3 = transT_v.rearrange("p (j i) -> p j i", j=T)

    def score_col(s):
        return scores_t[:, s * T : (s + 1) * T]

    # ---- forward chain ----
    V.tensor_tensor(score_col(0), em_t[:, 0:T], start_v, op=add)
    V.drain()
    for s in range(1, n_steps):
        sc_bc = score_col(s - 1).unsqueeze(1).broadcast_to([PB, T, T])
        V.tensor_tensor(cand3, sc_bc, transT3, op=add)
        V.tensor_reduce(best_t[:], cand3, axis=X, op=amax)  # TT->TR adjacency is safe
        V.drain()
        V.tensor_tensor(score_col(s), best_t[:], em_t[:, s * T : (s + 1) * T], op=add)
        V.drain()

    # ---- final argmax ----
    V.tensor_tensor(fs_t[:], score_col(n_steps - 1), end_v, op=add)
    V.drain()
    V.tensor_reduce(fbest_t[:], fs_t[:], axis=X, op=amax)
    V.drain()
    fbest_bc = fbest_t[:].broadcast_to([PB, T])
    V.tensor_tensor(seltrash_t[:], fs_t[:], fbest_bc, op=aeq)
    V.drain()
    V.tensor_tensor(mv20_t[:], seltrash_t[:], revJ_v, op=amult)
    V.drain()
    V.tensor_reduce(revtag_t[:, S - 1 : S], mv20_t[:], axis=X, op=amax)
    V.drain()

    # ---- batched backpointer extraction ----
    # for chunk of K steps starting at s0: recompute cand from scores (bit-exact),
    # grouped max, eq-mask, *revIota, grouped max -> rev-encoded bp.
    # bp for step s lands at scores col (s-1) (those scores are dead afterwards).
    if n_steps == S and (phases & 2):
        n_chunks = (S - 1 + KEXT - 1) // KEXT
    else:
        n_chunks = 0
    for c in range(n_chunks):
        s0 = 1 + c * KEXT
        k = min(KEXT, S - s0)
        candB4c = candB_t[:, : k * F].rearrange("p (k j i) -> p k j i", k=k, j=T)
        sc_blk = (
            scores_t[:, (s0 - 1) * T : (s0 - 1 + k) * T]
            .rearrange("p (k i) -> p k i", k=k)
            .unsqueeze(2)
            .broadcast_to([PB, k, T, T])
        )
        tr_bc = transT3.unsqueeze(1).broadcast_to([PB, k, T, T])
        V.tensor_tensor(candB4c, sc_blk, tr_bc, op=add)
        bestB3c = bestB_t[:, : k * T].rearrange("p (k j) -> p k j", k=k)
        V.tensor_reduce(bestB3c, candB4c, axis=X, op=amax)  # TT->TR safe
        V.drain()
        bb_bc = bestB3c.unsqueeze(3).broadcast_to([PB, k, T, T])
        V.tensor_tensor(candB4c, candB4c, bb_bc, op=aeq)  # in-place mask
        V.drain()
        rev_bc = (
            revIotaF_v.rearrange("p (j i) -> p j i", j=T)
            .unsqueeze(1)
            .broadcast_to([PB, k, T, T])
        )
        V.tensor_tensor(candB4c, candB4c, rev_bc, op=amult)  # in-place mv
        bp_out = scores_t[:, (s0 - 1) * T : (s0 - 1 + k) * T].rearrange(
            "p (k j) -> p k j", k=k
        )
        V.tensor_reduce(bp_out, candB4c, axis=X, op=amax)  # TT->TR safe
        V.drain()

    # ---- backward ----
    if n_steps == S and (phases & 4):
        for s in range(S - 1, 0, -1):
            V.scalar_tensor_tensor(
                seltrash_t[:],
                revJ_v,
                revtag_t[:, s : s + 1],
                bp_t[:, (s - 1) * T : s * T],
                op0=aeq,
                op1=amult,
                accum_out=revtag_t[:, s - 1 : s],
            )
            V.drain()

    # ---- decode ----
    V.tensor_scalar(tags_t[:], revtag_t[:], -1.0, float(REV_MAX), op0=amult, op1=add)

    nc.all_engine_barrier()
    nc.sync.dma_start(out_d[:], tags_t[:]).then_inc(dma_sem, 16)
    for eng in nc.engines.values():
        eng.wait_ge(dma_sem, 48)

    return nc


def _get_compiled():
    if "nc" not in _CACHE:
        _CACHE["nc"] = _build_nc()
    return _CACHE["nc"]


def _make_consts(start_transitions, end_transitions, transitions):
    transT = np.ascontiguousarray(transitions.astype(np.float32).T).reshape(1, F)
    revIotaF = np.tile((REV_MAX - np.arange(T, dtype=np.float32)), T).reshape(1, F)
    revJ = (REV_MAX - np.arange(T, dtype=np.float32)).reshape(1, T)
    cst = np.concatenate(
        [
            transT,
            revIotaF,
            revJ,
            start_transitions.astype(np.float32).reshape(1, T),
            end_transitions.astype(np.float32).reshape(1, T),
        ],
        axis=1,
    )
    return np.ascontiguousarray(np.broadcast_to(cst, (PB, cst.shape[1])))


def kernel(emissions, start_transitions, end_transitions, transitions):
    from concourse.bass_utils import run_bass_kernel_spmd

    emissions = np.asarray(emissions, dtype=np.float32)
    cst = _make_consts(
        np.asarray(start_transitions),
        np.asarray(end_transitions),
        np.asarray(transitions),
    )

    nc = _get_compiled()
    in_maps = []
    for c in range(NCORES):
        in_maps.append(
            {
                "em": np.ascontiguousarray(emissions[c * PB : (c + 1) * PB]),
                "cst": cst,
            }
        )
    res = run_bass_kernel_spmd(nc, in_maps, core_ids=list(range(NCORES)))
    out = np.concatenate([r["out"] for r in res.results], axis=0)
    return out.astype(np.int32)


# revision 12
# speedup vs baseline: 10.3589x; 1.3869x over previous
"""CRF Viterbi decode kernel for Trainium2 (8 NeuronCores, data-parallel over batch).

emissions [1024,1024,20] f32 + transitions -> best tag path [1024,1024] int32,
bit-exact with the jax reference (same f32 op order, first-index argmax ties).

Environment characteristics (measured): every engine instruction costs ~29us
regardless of size; engines do not overlap; an explicit DVE drain (~10us) is
required between a producer and a dependent consumer except for the TT->TR
pair. The design therefore minimizes instruction count:

  forward chain (per step, sequential):  TT cand / TR best / drain /
                                         TT score' / drain            (5 instr)
  backpointer extraction: recomputed in batches of K steps from the stored
    score history (5 ops + 3 drains per K steps) - bit-identical recompute.
  backward: one fused select-accumulate STT + drain per step.
"""

import sys

for _p in ("/opt/trn_rl_repo", "/root/.axon_site/_ro/trn_rl_repo"):
    import os as _os

    if _os.path.isdir(_p) and _p not in sys.path:
        sys.path.insert(0, _p)

import numpy as np

B, S, T = 1024, 1024, 20
NCORES = 8
PB = B // NCORES  # 128
F = T * T  # 400
REV_MAX = T - 1  # 19
KEXT = 16  # extraction batch size (steps)

_CACHE = {}


def _build_nc(n_steps=None, phases=7):
    import concourse.bass as bass
    import concourse.mybir as mybir

    if n_steps is None:
        n_steps = S
    nc = bass.Bass("TRN2", debug=False, num_devices=NCORES)
    f32 = mybir.dt.float32
    i32 = mybir.dt.int32
    add = mybir.AluOpType.add
    amax = mybir.AluOpType.max
    aeq = mybir.AluOpType.is_equal
    amult = mybir.AluOpType.mult
    X = mybir.AxisListType.X

    NC_CONST = F + F + T + T + T
    em_d = nc.dram_tensor("em", [PB, S, T], f32, kind="ExternalInput").ap()
    cst_d = nc.dram_tensor("cst", [PB, NC_CONST], f32, kind="ExternalInput").ap()
    out_d = nc.dram_tensor("out", [PB, S], i32, kind="ExternalOutput").ap()

    def sb(name, shape, dt):
        return nc.alloc_sbuf_tensor(name, shape, dt).ap()

    em_t = sb("em_sb", [PB, S * T], f32)            # 80 KB/partition
    cst_t = sb("cst_sb", [PB, NC_CONST], f32)
    scores_t = sb("scores_sb", [PB, S * T], f32)    # 80 KB/partition: score_s at col s*T
    cand_t = sb("cand_sb", [PB, F], f32)
    best_t = sb("best_sb", [PB, T], f32)
    candB_t = sb("candB_sb", [PB, KEXT * F], f32)   # extraction batch (in-place reuse)
    bestB_t = sb("bestB_sb", [PB, KEXT * T], f32)
    bp_t = scores_t  # bp for step s overwrites score col (s-1) after extraction
    fs_t = sb("fs_sb", [PB, T], f32)
    fbest_t = sb("fbest_sb", [PB, 1], f32)
    revtag_t = sb("revtag_sb", [PB, S], f32)
    seltrash_t = sb("seltrash_sb", [PB, T], f32)
    mv20_t = sb("mv20_sb", [PB, T], f32)
    tags_t = sb("tags_sb", [PB, S], i32)

    transT_v = cst_t[:, 0:F]
    revIotaF_v = cst_t[:, F : 2 * F]
    revJ_v = cst_t[:, 2 * F : 2 * F + T]
    start_v = cst_t[:, 2 * F + T : 2 * F + 2 * T]
    end_v = cst_t[:, 2 * F + 2 * T : 2 * F + 3 * T]

    V = nc.vector

    dma_sem = nc.alloc_semaphore()
    nc.sync.dma_start(em_t[:], em_d.rearrange("b s t -> b (s t)")).then_inc(dma_sem, 16)
    nc.sync.dma_start(cst_t[:], cst_d[:]).then_inc(dma_sem, 16)
    V.wait_ge(dma_sem, 32)

    cand3 = cand_t[:].rearrange("p (j i) -> p j i", j=T)
    transT3 = transT_v.rearrange("p (j i) -> p j i", j=T)

    def score_col(s):
        return scores_t[:, s * T : (s + 1) * T]

    # ---- forward chain ----
    V.tensor_tensor(score_col(0), em_t[:, 0:T], start_v, op=add)
    V.drain()
    for s in range(1, n_steps):
        sc_bc = score_col(s - 1).unsqueeze(1).broadcast_to([PB, T, T])
        V.tensor_tensor(cand3, sc_bc, transT3, op=add)
        V.tensor_reduce(best_t[:], cand3, axis=X, op=amax)  # TT->TR adjacency is safe
        V.drain()
        V.tensor_tensor(score_col(s), best_t[:], em_t[:, s * T : (s + 1) * T], op=add)
        V.drain()

    # ---- final argmax ----
    V.tensor_tensor(fs_t[:], score_col(n_steps - 1), end_v, op=add)
    V.drain()
    V.tensor_reduce(fbest_t[:], fs_t[:], axis=X, op=amax)
    V.drain()
    fbest_bc = fbest_t[:].broadcast_to([PB, T])
    V.tensor_tensor(seltrash_t[:], fs_t[:], fbest_bc, op=aeq)
    V.drain()
    V.tensor_tensor(mv20_t[:], seltrash_t[:], revJ_v, op=amult)
    V.drain()
    V.tensor_reduce(revtag_t[:, S - 1 : S], mv20_t[:], axis=X, op=amax)
    V.drain()

    # ---- batched backpointer extraction ----
    # for chunk of K steps starting at s0: recompute cand from scores (bit-exact),
    # grouped max, eq-mask, *revIota, grouped max -> rev-encoded bp.
    # bp for step s lands at scores col (s-1) (those scores are dead afterwards).
    if n_steps == S and (phases & 2):
        n_chunks = (S - 1 + KEXT - 1) // KEXT
    else:
        n_chunks = 0
    for c in range(n_chunks):
        s0 = 1 + c * KEXT
        k = min(KEXT, S - s0)
        candB4c = candB_t[:, : k * F].rearrange("p (k j i) -> p k j i", k=k, j=T)
        sc_blk = (
            scores_t[:, (s0 - 1) * T : (s0 - 1 + k) * T]
            .rearrange("p (k i) -> p k i", k=k)
            .unsqueeze(2)
            .broadcast_to([PB, k, T, T])
        )
        tr_bc = transT3.unsqueeze(1).broadcast_to([PB, k, T, T])
        V.tensor_tensor(candB4c, sc_blk, tr_bc, op=add)
        bestB3c = bestB_t[:, : k * T].rearrange("p (k j) -> p k j", k=k)
        V.tensor_reduce(bestB3c, candB4c, axis=X, op=amax)  # TT->TR safe
        V.drain()
        bb_bc = bestB3c.unsqueeze(3).broadcast_to([PB, k, T, T])
        V.tensor_tensor(candB4c, candB4c, bb_bc, op=aeq)  # in-place mask
        V.drain()
        rev_bc = (
            revIotaF_v.rearrange("p (j i) -> p j i", j=T)
            .unsqueeze(1)
            .broadcast_to([PB, k, T, T])
        )
        V.tensor_tensor(candB4c, candB4c, rev_bc, op=amult)  # in-place mv
        bp_out = scores_t[:, (s0 - 1) * T : (s0 - 1 + k) * T].rearrange(
            "p (k j) -> p k j", k=k
        )
        V.tensor_reduce(bp_out, candB4c, axis=X, op=amax)  # TT->TR safe
        V.drain()

    # ---- backward: blocked pointer composition (exact integer selects) ----
    # Positions 0..S-1 in NB blocks of LB. Phase 1 composes each block's LB
    # backpointer maps into C_blk (batched over blocks). Phase 2 walks the NB
    # block boundaries serially. Phase 3 regenerates interior positions,
    # batched over blocks. A virtual identity bp for step S (written into the
    # dead scores col S-1) makes all strides uniform.
    if n_steps == S and (phases & 4):
        LB = 16
        NB = S // LB   # 64
        GB = 32        # blocks per instruction group (scratch size limit)
        comp_t = sb("comp_sb", [PB, NB * T], f32)
        zero20_t = sb("zero20_sb", [PB, T], f32)
        scr = em_t  # emissions are dead now; reuse as [PB, GB*F] scratch

        V.memset(zero20_t[:], 0.0)
        V.drain()
        # identity map (rev space) at scores col S-1: bp_S[m] = rev(m)
        V.tensor_tensor(scores_t[:, (S - 1) * T : S * T], revJ_v, zero20_t[:], op=add)
        # comp := identity for all blocks
        compNB = comp_t[:].rearrange("p (b j) -> p b j", b=NB)
        V.tensor_tensor(
            compNB,
            revJ_v.unsqueeze(1).broadcast_to([PB, NB, T]),
            zero20_t[:].unsqueeze(1).broadcast_to([PB, NB, T]),
            op=add,
        )
        V.drain()

        bpB = scores_t[:].rearrange("p (b r) -> p b r", b=NB)  # blocks of LB*T cols
        rtB = revtag_t[:].rearrange("p (b r) -> p b r", b=NB)  # blocks of LB cols

        # ---- phase 1 ----
        # iteration k applies bp at step (blk+1)*LB - k  (k = 0..LB-1);
        # within-block col offset (LB-k-1)*T. comp'[j] = bp[comp[j]].
        for k in range(LB):
            off = (LB - k - 1) * T
            for g in range(NB // GB):
                b0 = g * GB
                scr4 = scr[:, 0 : GB * F].rearrange(
                    "p (b j m) -> p b j m", b=GB, j=T
                )
                compg = compNB[:, b0 : b0 + GB]
                V.drain()
                V.tensor_tensor(
                    scr4,
                    compg.unsqueeze(3).broadcast_to([PB, GB, T, T]),
                    revJ_v.unsqueeze(1).unsqueeze(1).broadcast_to([PB, GB, T, T]),
                    op=aeq,
                )
                V.drain()
                bsl = bpB[:, b0 : b0 + GB, off : off + T]  # [P, GB, T]
                V.tensor_tensor(
                    scr4,
                    scr4,
                    bsl.unsqueeze(2).broadcast_to([PB, GB, T, T]),
                    op=amult,
                )
                V.tensor_reduce(compg, scr4, axis=X, op=add)
        V.drain()

        # ---- phase 2: boundary walk (serial) ----
        for blk in range(NB - 1, -1, -1):
            src_col = S - 1 if blk == NB - 1 else (blk + 1) * LB
            V.scalar_tensor_tensor(
                seltrash_t[:],
                revJ_v,
                revtag_t[:, src_col : src_col + 1],
                comp_t[:, blk * T : (blk + 1) * T],
                op0=aeq,
                op1=amult,
                accum_out=revtag_t[:, blk * LB : blk * LB + 1],
            )
            V.drain()

        # ---- phase 3: interior positions ----
        # iteration k (0..LB-2) fills position (blk+1)*LB - 1 - k from source
        # position (blk+1)*LB - k via bp step (blk+1)*LB - k.
        for k in range(LB - 1):
            nblk_k = NB - 1 if k == 0 else NB  # position S-1 already known
            for b0 in range(0, nblk_k, GB):
                nb = min(GB, nblk_k - b0)
                scr3 = scr[:, 0 : nb * T].rearrange(
                    "p (b j m) -> p b j m", b=nb, j=1
                )
                if k == 0:
                    # source cols (blk+1)*LB for blk = 0..NB-2: shifted view
                    srcv = (
                        revtag_t[:, LB:]
                        .rearrange("p (b r) -> p b r", b=NB - 1)[
                            :, b0 : b0 + nb, 0:1
                        ]
                    )
                else:
                    srcv = rtB[:, b0 : b0 + nb, LB - k : LB - k + 1]
                bsl = bpB[:, b0 : b0 + nb, (LB - k - 1) * T : (LB - k) * T]
                V.drain()
                V.tensor_tensor(
                    scr3,
                    srcv.unsqueeze(2).broadcast_to([PB, nb, 1, T]),
                    revJ_v.unsqueeze(1).unsqueeze(1).broadcast_to([PB, nb, 1, T]),
                    op=aeq,
                )
                V.drain()
                V.tensor_tensor(
                    scr3,
                    scr3,
                    bsl.unsqueeze(2).broadcast_to([PB, nb, 1, T]),
                    op=amult,
                )
                V.tensor_reduce(
                    rtB[:, b0 : b0 + nb, LB - 1 - k : LB - k], scr3, axis=X, op=add
                )
        V.drain()

    # ---- decode ----
    V.tensor_scalar(tags_t[:], revtag_t[:], -1.0, float(REV_MAX), op0=amult, op1=add)

    nc.all_engine_barrier()
    nc.sync.dma_start(out_d[:], tags_t[:]).then_inc(dma_sem, 16)
    for eng in nc.engines.values():
        eng.wait_ge(dma_sem, 48)

    return nc


def _get_compiled():
    if "nc" not in _CACHE:
        _CACHE["nc"] = _build_nc()
    return _CACHE["nc"]


def _make_consts(start_transitions, end_transitions, transitions):
    transT = np.ascontiguousarray(transitions.astype(np.float32).T).reshape(1, F)
    revIotaF = np.tile((REV_MAX - np.arange(T, dtype=np.float32)), T).reshape(1, F)
    revJ = (REV_MAX - np.arange(T, dtype=np.float32)).reshape(1, T)
    cst = np.concatenate(
        [
            transT,
            revIotaF,
            revJ,
            start_transitions.astype(np.float32).reshape(1, T),
            end_transitions.astype(np.float32).reshape(1, T),
        ],
        axis=1,
    )
    return np.ascontiguousarray(np.broadcast_to(cst, (PB, cst.shape[1])))


def kernel(emissions, start_transitions, end_transitions, transitions):
    from concourse.bass_utils import run_bass_kernel_spmd

    emissions = np.asarray(emissions, dtype=np.float32)
    cst = _make_consts(
        np.asarray(start_transitions),
        np.asarray(end_transitions),
        np.asarray(transitions),
    )

    nc = _get_compiled()
    in_maps = []
    for c in range(NCORES):
        in_maps.append(
            {
                "em": np.ascontiguousarray(emissions[c * PB : (c + 1) * PB]),
                "cst": cst,
            }
        )
    res = run_bass_kernel_spmd(nc, in_maps, core_ids=list(range(NCORES)))
    out = np.concatenate([r["out"] for r in res.results], axis=0)
    return out.astype(np.int32)
